# revision 51
# baseline (speedup 1.0000x reference)
"""Trainium2 Bass kernel for nn_NodeUpdate (GNN message passing node update).

Strategy (8-core SPMD, graph/data parallel by destination node):
  - Host shards edges by destination node (6250 dests/core) and, within each
    destination segment, orders edges so the reference argmin edge sits at
    slot 0 and the reference argmax edge at slot len-1. Destinations are
    sorted by degree and grouped into 128-dest blocks; each block gets its
    own padded segment length L (max cap in block, even, min 4; pads are
    copies of the argmax edge, so min/max stay exact and the sum is corrected
    by -pad*maxrow). This keeps total padding ~6% and makes every segment
    reduction a fixed-length contiguous reduction with static argmin/argmax
    row slices.
  - Device (per core): streams the padded fp16 edge buffer (SWDGE queue),
    computes per-dest min/max/sum via contiguous halving trees on the Vector
    engine (fp16 2x mode), corrects the sum for padding, assembles the
    440-wide stats block X, writes it to DRAM on the SWDGE queue (so edge
    prefetch and X writes never queue behind span-side DMAs), reads X back
    with xbar-transpose DMAs on the sync ring (1024-dest spans) to form X^T,
    and runs the 3-layer MLP with weights stationary, activations kept
    transposed [out_ch, dests]. The X DRAM buffer is an ExternalInput
    pre-filled by the host with the node features x at cols 440:472 and
    zeros elsewhere, so the transposed readback picks up x directly and no
    on-device merge or zero-fill DMA is needed. MLP chunks are emitted one span behind
    their transposes and stage-interleaved across a span's two 512-dest
    chunks, so Scalar RELUs of one chunk overlap the other chunk's matmuls
    and the PE never stalls on in-flight transposes. Tiny keep-warm matmuls
    prevent PE p-state downclocking between MLP bursts.
  - Host gathers per-core [32, S_pad] outputs, un-permutes to [50000, 32].
"""

import os
import time
from contextlib import ExitStack

import numpy as np

F16 = np.float16

N_NODES = 50000
N_EDGES = 800000
EC = 88
NC_DIM = 32
HC = 256
OC = 32
N_CORES = 8
S = N_NODES // N_CORES  # 6250
TARGET_GL = 160         # target dests-per-partition * L per super-tile
SPAN = 1024             # dests per transpose span

_LAST = {}  # debug/profiling stash for test.py


# ----------------------------------------------------------------------------
# Host-side preparation
# ----------------------------------------------------------------------------

def _host_prep(edge_index, edge_attr):
    dest = np.asarray(edge_index[1], np.int64)
    key = np.ascontiguousarray(np.asarray(edge_attr, np.float32)[:, 0])
    E = dest.shape[0]
    counts = np.bincount(dest, minlength=N_NODES)
    assert counts.min() >= 1, "empty destination segments not supported"

    order = np.argsort(dest, kind="stable")  # within dest: eid ascending
    starts = np.concatenate([[0], np.cumsum(counts)[:-1]]).astype(np.int64)
    k_s = key[order]
    segmax = np.maximum.reduceat(k_s, starts)
    segmin = np.minimum.reduceat(k_s, starts)
    dop = np.repeat(np.arange(N_NODES), counts)
    pos = np.arange(E, dtype=np.int64)
    argmax_pos = np.minimum.reduceat(np.where(k_s == segmax[dop], pos, E), starts)
    argmin_pos = np.minimum.reduceat(np.where(k_s == segmin[dop], pos, E), starts)

    cap = counts + (argmax_pos == argmin_pos)

    # Block-sorted binning: each core sorts its dests by cap ascending, pads
    # to a whole number of 128-dest blocks (pads first, cheapest L), and each
    # block gets its own L = max cap in block (across cores, since all cores
    # share one compiled program), rounded up to even, min 4.
    NB = (S + 127) // 128              # blocks per core
    S_pad = NB * 128
    npad_dests = S_pad - S
    plans = []
    block_max = np.zeros((N_CORES, NB), np.int64)
    for c in range(N_CORES):
        d0 = c * S
        dests = np.arange(d0, d0 + S)
        srt = dests[np.argsort(cap[d0:d0 + S], kind="stable")]
        slots = np.concatenate([np.full(npad_dests, -1, np.int64), srt])
        plans.append(slots)
        caps_sorted = np.concatenate(
            [np.zeros(npad_dests, np.int64), np.sort(cap[d0:d0 + S])])
        block_max[c] = caps_sorted.reshape(NB, 128).max(axis=1)

    L_of_block = np.maximum(4, ((block_max.max(axis=0) + 1) // 2) * 2)
    # runs of equal L -> bins
    L_values, n_b = [], []
    for L in L_of_block:
        if L_values and L_values[-1] == L:
            n_b[-1] += 128
        else:
            L_values.append(int(L))
            n_b.append(128)
    n_b = np.array(n_b, np.int64)
    E_pad = int(sum(L * n for L, n in zip(L_values, n_b)))

    return dict(L_values=L_values, n_b=n_b, S_pad=S_pad, E_pad=E_pad,
                order=order, starts=starts, counts=counts,
                argmax_pos=argmax_pos, argmin_pos=argmin_pos, plans=plans)


def _build_core_arrays(c, meta, x, edge_attr):
    L_values, n_b = meta["L_values"], meta["n_b"]
    order, starts, counts = meta["order"], meta["starts"], meta["counts"]
    argmax_pos, argmin_pos = meta["argmax_pos"], meta["argmin_pos"]
    dest_slots = meta["plans"][c]
    S_pad, E_pad = meta["S_pad"], meta["E_pad"]

    Ls = np.concatenate([np.full(n, L, np.int64) for L, n in zip(L_values, n_b)])
    row_start = np.concatenate([[0], np.cumsum(Ls)[:-1]])

    gidx = np.full(E_pad, -1, np.int64)
    real = dest_slots >= 0
    d_real = dest_slots[real]
    rs_real = row_start[real]
    L_real = Ls[real]
    cnt = counts[d_real]
    st = starts[d_real]
    apos = (argmax_pos[d_real] - st).astype(np.int64)
    ipos = (argmin_pos[d_real] - st).astype(np.int64)

    for i in range(d_real.shape[0]):
        s = st[i]; n = cnt[i]; L = L_real[i]; r0 = rs_real[i]
        seg = order[s:s + n]
        ia, ii = apos[i], ipos[i]
        if ia == ii:
            slots = np.concatenate([seg[ii:ii + 1], np.delete(seg, ii)])
        else:
            slots = np.concatenate(
                [seg[ii:ii + 1], np.delete(seg, [min(ia, ii), max(ia, ii)]),
                 seg[ia:ia + 1]])
        gidx[r0:r0 + n] = slots
        gidx[r0 + n:r0 + L] = seg[ia]

    edges = np.zeros((E_pad, EC), F16)
    valid = gidx >= 0
    edges[valid] = edge_attr[gidx[valid]].astype(F16)

    lens = np.where(real, counts[np.maximum(dest_slots, 0)], Ls).astype(np.float32)
    neg_pad = -(Ls - lens).astype(np.float32)
    neg_pad[~real] = 0.0
    inv_len = (1.0 / lens).astype(np.float32)

    xdram0 = np.zeros((S_pad, 512), F16)
    xdram0[real, 440:440 + NC_DIM] = np.asarray(x, np.float32)[d_real].astype(F16)
    NT = S_pad // 128

    return dict(
        edges=edges,
        negpad=np.ascontiguousarray(
            np.broadcast_to(neg_pad[:, None], (S_pad, EC)).reshape(
                NT, 128, EC).transpose(1, 0, 2)).astype(F16),
        invlen=np.ascontiguousarray(
            np.broadcast_to(inv_len[:, None], (S_pad, EC)).reshape(
                NT, 128, EC).transpose(1, 0, 2)).astype(F16),
        xdram0=xdram0,
        dest_slots=dest_slots,
    )


def _pack_weights(W1, b1, W2, b2, W3, b3):
    W1 = np.asarray(W1, np.float32); W2 = np.asarray(W2, np.float32)
    W3 = np.asarray(W3, np.float32)
    b1 = np.asarray(b1, np.float32); b2 = np.asarray(b2, np.float32)
    b3 = np.asarray(b3, np.float32)
    # reorder W1 rows: stats first (minrow maxrow mn mean mx), then x
    W1r = np.concatenate([W1[NC_DIM:], W1[:NC_DIM]], axis=0)  # [472, 256]
    W1p = np.zeros((512, HC), np.float32)
    W1p[:472] = W1r
    w1 = np.ascontiguousarray(
        W1p.reshape(4, 128, HC).transpose(1, 0, 2)).astype(F16)   # [128,4,256]
    w2 = np.ascontiguousarray(
        W2.reshape(2, 128, HC).transpose(1, 0, 2)).astype(F16)    # [128,2,256]
    w3 = np.ascontiguousarray(
        W3.reshape(2, 128, OC).transpose(1, 0, 2)).astype(F16)    # [128,2,32]
    b1p = np.ascontiguousarray(b1.reshape(2, 128).T)               # [128,2]
    b2p = np.ascontiguousarray(b2.reshape(2, 128).T)
    b3p = np.zeros((128, 1), np.float32)
    b3p[:OC, 0] = b3
    ident = np.eye(128, dtype=F16)
    return dict(w1=w1, w2=w2, w3=w3, b1=b1p, b2=b2p, b3=b3p, ident=ident)


# ----------------------------------------------------------------------------
# Device program
# ----------------------------------------------------------------------------

def _super_tiles(meta):
    """Static schedule: list of (L, G, dest0, row0)."""
    sts = []
    dest0 = 0
    row0 = 0
    for L, n in zip(meta["L_values"], meta["n_b"]):
        G_full = max(1, TARGET_GL // L)
        blocks = n // 128
        pos = 0
        while pos < blocks:
            g = min(G_full, blocks - pos)
            # graduated pipeline ramp: tiny first tiles so the Vector engine
            # starts as soon as possible instead of waiting on one big DMA
            if len(sts) < 4:
                g = min(g, 1 << len(sts))
            sts.append((L, g, dest0 + pos * 128, row0 + pos * 128 * L))
            pos += g
        dest0 += n
        row0 += n * L
    return sts


def _build_program(meta):
    import concourse.bass as bass
    import concourse.tile as tile
    from concourse import bacc, mybir

    f32 = mybir.dt.float32
    f16 = mybir.dt.float16
    Alu = mybir.AluOpType
    Act = mybir.ActivationFunctionType

    S_pad, E_pad = meta["S_pad"], meta["E_pad"]
    NT = S_pad // 128
    NBLK = S_pad // 128
    sts = _super_tiles(meta)

    nc = bacc.Bacc("TRN2", target_bir_lowering=False, debug=False,
                   num_devices=N_CORES)

    edges_h = nc.dram_tensor("edges", [E_pad, EC], f16, kind="ExternalInput")
    np_h = nc.dram_tensor("negpad", [128, NT, EC], f16, kind="ExternalInput")
    il_h = nc.dram_tensor("invlen", [128, NT, EC], f16, kind="ExternalInput")
    w1_h = nc.dram_tensor("w1", [128, 4, HC], f16, kind="ExternalInput")
    w2_h = nc.dram_tensor("w2", [128, 2, HC], f16, kind="ExternalInput")
    w3_h = nc.dram_tensor("w3", [128, 2, OC], f16, kind="ExternalInput")
    b1_h = nc.dram_tensor("b1", [128, 2], f32, kind="ExternalInput")
    b2_h = nc.dram_tensor("b2", [128, 2], f32, kind="ExternalInput")
    b3_h = nc.dram_tensor("b3", [128, 1], f32, kind="ExternalInput")
    xdram_h = nc.dram_tensor("xdram", [S_pad, 512], f16, kind="ExternalInput")
    outT_h = nc.dram_tensor("outT", [OC, S_pad], f32, kind="ExternalOutput")

    with tile.TileContext(nc) as tc, ExitStack() as ctx:
        singles = ctx.enter_context(tc.tile_pool(name="singles", bufs=1))
        epool = ctx.enter_context(tc.tile_pool(name="edg", bufs=3))
        tpool = ctx.enter_context(tc.tile_pool(name="tree", bufs=1))
        xpool = ctx.enter_context(tc.tile_pool(name="xrows", bufs=2))
        xtp = ctx.enter_context(tc.tile_pool(name="xtq", bufs=3))
        hp = ctx.enter_context(tc.tile_pool(name="hid", bufs=3))
        opool = ctx.enter_context(tc.tile_pool(name="outb", bufs=2))
        pmm = ctx.enter_context(tc.tile_pool(name="psmm", bufs=4, space="PSUM"))
        pwarm = ctx.enter_context(tc.tile_pool(name="pswm", bufs=1, space="PSUM"))

        # --- load constants (npad/ilen first: the first tile's mean ops
        # need them; scalar ring so they don't queue behind weights) ---
        npad_sb = singles.tile([128, NT, EC], f16)
        nc.scalar.dma_start(out=npad_sb, in_=np_h.ap())
        ilen_sb = singles.tile([128, NT, EC], f16)
        nc.scalar.dma_start(out=ilen_sb, in_=il_h.ap())
        w1_sb = singles.tile([128, 4, HC], f16)
        nc.sync.dma_start(out=w1_sb, in_=w1_h.ap())
        w2_sb = singles.tile([128, 2, HC], f16)
        nc.sync.dma_start(out=w2_sb, in_=w2_h.ap())
        w3_sb = singles.tile([128, 2, OC], f16)
        nc.sync.dma_start(out=w3_sb, in_=w3_h.ap())
        b1_sb = singles.tile([128, 2], f32)
        nc.sync.dma_start(out=b1_sb, in_=b1_h.ap())
        b2_sb = singles.tile([128, 2], f32)
        nc.sync.dma_start(out=b2_sb, in_=b2_h.ap())
        b3_sb = singles.tile([128, 1], f32)
        nc.sync.dma_start(out=b3_sb, in_=b3_h.ap())

        def bcast(ap2d, n):
            return bass.AP(tensor=ap2d.tensor, offset=ap2d.offset,
                           ap=[*ap2d.ap, [0, n]])

        def emit_tree(et4, L, G, op, final_out):
            h = L // 2
            T = tpool.tile([128, G, h * EC], f16, tag="tree", name="tree")
            T4 = T[:].rearrange("p g (m e) -> p g m e", e=EC)
            nc.vector.tensor_tensor(out=T4[:, :, 0:h, :], in0=et4[:, :, 0:h, :],
                                    in1=et4[:, :, h:2 * h, :], op=op)
            m = h
            while m > 1:
                if m % 2 == 1:
                    nc.vector.tensor_tensor(out=T4[:, :, 0, :],
                                            in0=T4[:, :, 0, :],
                                            in1=T4[:, :, m - 1, :], op=op)
                    m -= 1
                h2 = m // 2
                if h2 == 1:
                    nc.vector.tensor_tensor(out=final_out, in0=T4[:, :, 0, :],
                                            in1=T4[:, :, 1, :], op=op)
                else:
                    nc.vector.tensor_tensor(out=T4[:, :, 0:h2, :],
                                            in0=T4[:, :, 0:h2, :],
                                            in1=T4[:, :, h2:m, :], op=op)
                m = h2

        blocks_ready = 0
        spans_emitted = 0
        # span boundaries: SPAN-sized, but carve the final 256 dests into a
        # tiny last span so the pipeline tail is short
        bounds = [0, 512, 1024] + list(range(1024 + SPAN, S_pad, SPAN)) + [S_pad]
        bounds = sorted(set(b for b in bounds if b <= S_pad))
        if S_pad - bounds[-2] > 256:
            bounds.insert(-1, S_pad - 256)
        spans = [(bounds[i], bounds[i + 1] - bounds[i])
                 for i in range(len(bounds) - 1)]
        NSPAN = len(spans)
        xt_span = [None] * 4
        span_d0 = [0]
        pending_spans = []

        def emit_span(si):
            d0, nd = spans[si]
            parts = [128, 128, 128, 88]
            for k in range(4):
                xt_t = xtp.tile([128, SPAN], f16, tag=f"xt{k}", name=f"xt{k}")
                nc.sync.dma_start(
                    out=xt_t[:, 0:nd],
                    in_=xdram_h.ap()[d0:d0 + nd, k * 128:(k + 1) * 128],
                    transpose=True)
                xt_span[k] = xt_t
            span_d0[0] = d0

        def emit_span_chunks(sd0, snd, xts):
            """MLP for one span, stage-interleaved across its 512-dest chunks
            so each chunk's RELUs overlap the other chunk's matmuls."""
            parts = [128, 128, 128, 88]
            chunks = [(sd0 + coff, min(512, snd - coff))
                      for coff in range(0, snd, 512)]
            h1s_of, h2s_of = {}, {}
            for (cd0, N) in chunks:
                off = cd0 - sd0
                h1s = []
                for cc in range(2):
                    ps = pmm.tile([128, 512], f32, tag="mm", name="mm")
                    for k in range(4):
                        P = parts[k]
                        nc.tensor.matmul(ps[:, 0:N],
                                         w1_sb[0:P, k, cc * 128:(cc + 1) * 128],
                                         xts[k][0:P, off:off + N],
                                         start=(k == 0), stop=(k == 3))
                    h1 = hp.tile([128, 512], f16, tag=f"h1{cc}", name=f"h1{cc}")
                    nc.scalar.activation(out=h1[:, 0:N], in_=ps[:, 0:N],
                                         func=Act.Relu, bias=b1_sb[:, cc:cc + 1],
                                         scale=1.0)
                    h1s.append(h1)
                h1s_of[cd0] = h1s
            for (cd0, N) in chunks:
                h1s = h1s_of[cd0]
                h2s = []
                for cc in range(2):
                    ps = pmm.tile([128, 512], f32, tag="mm", name="mm")
                    for k in range(2):
                        nc.tensor.matmul(ps[:, 0:N],
                                         w2_sb[:, k, cc * 128:(cc + 1) * 128],
                                         h1s[k][:, 0:N],
                                         start=(k == 0), stop=(k == 1))
                    h2 = hp.tile([128, 512], f16, tag=f"h2{cc}", name=f"h2{cc}")
                    nc.scalar.activation(out=h2[:, 0:N], in_=ps[:, 0:N],
                                         func=Act.Relu, bias=b2_sb[:, cc:cc + 1],
                                         scale=1.0)
                    h2s.append(h2)
                h2s_of[cd0] = h2s
            for (cd0, N) in chunks:
                h2s = h2s_of[cd0]
                ps3 = pmm.tile([32, 512], f32, tag="mm3", name="mm3", bufs=2)
                for k in range(2):
                    nc.tensor.matmul(ps3[:, 0:N], w3_sb[:, k, :],
                                     h2s[k][:, 0:N], start=(k == 0), stop=(k == 1))
                ob = opool.tile([32, 512], f32, tag="ob", name="ob")
                nc.scalar.activation(out=ob[:, 0:N], in_=ps3[:, 0:N],
                                     func=Act.Identity, bias=b3_sb[0:OC, :],
                                     scale=1.0)
                nc.scalar.dma_start(out=outT_h.ap()[:, cd0:cd0 + N],
                                    in_=ob[:, 0:N])

        for (L, G, dest0, row0) in sts:
            et = epool.tile([128, G, L * EC], f16, tag="et", name="et")
            src_ap = edges_h.ap()[row0:row0 + 128 * G * L, :].rearrange(
                "(g p l) e -> p g (l e)", g=G, p=128)
            nc.gpsimd.dma_start(out=et, in_=src_ap)
            et4 = et[:].rearrange("p g (l e) -> p g l e", e=EC)

            dsl = xdram_h.ap()[dest0:dest0 + 128 * G, :]
            xt = xpool.tile([128, G, 440], f16, tag="xt", name="xt")
            # minrow / maxrow (slots 0 and L-1) copied on the Scalar engine
            nc.scalar.copy(out=xt[:, :, 0:EC], in_=et4[:, :, 0, :])
            nc.scalar.copy(out=xt[:, :, EC:2 * EC], in_=et4[:, :, L - 1, :])
            # trees: mn -> xt[176:264], mx -> xt[352:440]
            emit_tree(et4, L, G, Alu.min, xt[:, :, 2 * EC:3 * EC])
            emit_tree(et4, L, G, Alu.max, xt[:, :, 4 * EC:5 * EC])
            sm = tpool.tile([128, G, EC], f16, tag="sm", name="sm")
            emit_tree(et4, L, G, Alu.add, sm[:, :, :])
            # sum correction + mean -> xt[88:176] (all fp16, 2x mode)
            c0 = dest0 // 128
            t1 = tpool.tile([128, G, EC], f16, tag="t1", name="t1")
            nc.vector.tensor_tensor(out=t1[:, :, :], in0=et4[:, :, L - 1, :],
                                    in1=npad_sb[:, c0:c0 + G, :],
                                    op=Alu.mult)
            nc.vector.tensor_tensor(out=sm[:, :, :], in0=sm[:, :, :],
                                    in1=t1[:, :, :], op=Alu.add)
            nc.vector.tensor_tensor(out=xt[:, :, 3 * EC:4 * EC],
                                    in0=sm[:, :, :],
                                    in1=ilen_sb[:, c0:c0 + G, :],
                                    op=Alu.mult)
            # PE keep-warm pulses: tiny matmuls reading freshly written stats
            for woff in (0, EC, 2 * EC):
                wps = pwarm.tile([64, 88], f32, tag="warm", name="warm")
                nc.tensor.matmul(wps, xt[:, 0, woff:woff + 64],
                                 xt[:, 0, woff:woff + EC],
                                 start=True, stop=True)
            # X rows (cols 0:440) to X DRAM in one DMA (SWDGE queue, so it
            # never waits behind span transpose reads on the sync ring)
            nc.gpsimd.dma_start(
                out=dsl[:, 0:5 * EC].rearrange("(g p) c -> p g c", p=128),
                in_=xt)
            blocks_ready = dest0 // 128 + G
            while spans_emitted < NSPAN and \
                    (spans[spans_emitted][0] + spans[spans_emitted][1]
                     <= blocks_ready * 128):
                # transposes now; MLP chunks one span later, so the PE never
                # reaches a span's matmuls before its transposes complete
                emit_span(spans_emitted)
                if pending_spans:
                    sd0, snd, xts = pending_spans.pop(0)
                    emit_span_chunks(sd0, snd, xts)
                sd0, snd = spans[spans_emitted]
                pending_spans.append((sd0, snd, list(xt_span)))
                spans_emitted += 1
        for (sd0, snd, xts) in pending_spans:
            emit_span_chunks(sd0, snd, xts)

    nc.finalize()
    return nc


# ----------------------------------------------------------------------------
# Entry point
# ----------------------------------------------------------------------------

def kernel(x, edge_index, edge_attr, u, batch, W1, b1, W2, b2, W3, b3,
           **_unused):
    t0 = time.time()
    x = np.asarray(x)
    edge_index = np.asarray(edge_index)
    edge_attr = np.asarray(edge_attr, np.float32)

    meta = _host_prep(edge_index, edge_attr)
    packed = _pack_weights(W1, b1, W2, b2, W3, b3)
    cores = [_build_core_arrays(c, meta, x, edge_attr) for c in range(N_CORES)]
    t1 = time.time()

    nc = _build_program(meta)
    t2 = time.time()

    in_maps = []
    for c in range(N_CORES):
        m = dict(edges=cores[c]["edges"], xdram=cores[c]["xdram0"],
                 negpad=cores[c]["negpad"], invlen=cores[c]["invlen"],
                 w1=packed["w1"], w2=packed["w2"], w3=packed["w3"],
                 b1=packed["b1"], b2=packed["b2"], b3=packed["b3"])
        in_maps.append(m)

    from concourse.bass_utils import run_bass_kernel_spmd
    trace = os.environ.get("NNK_TRACE", "0") == "1"
    res = run_bass_kernel_spmd(nc, in_maps, core_ids=list(range(N_CORES)),
                               trace=trace)
    t3 = time.time()

    out = np.zeros((N_NODES, OC), np.float32)
    for c in range(N_CORES):
        outT = res.results[c]["outT"]
        ds = cores[c]["dest_slots"]
        real = ds >= 0
        out[ds[real]] = outT[:, real].T
    _LAST.update(nc=nc, meta=meta, in_maps=in_maps, res=res,
                 host_s=t1 - t0, build_s=t2 - t1, run_s=t3 - t2)
    return out



# revision 53
# speedup vs baseline: 1.0515x; 1.0515x over previous
"""Trainium2 Bass kernel for nn_NodeUpdate (GNN message passing node update).

Strategy (8-core SPMD, graph/data parallel by destination node):
  - Host shards edges by destination node (6250 dests/core) and, within each
    destination segment, orders edges so the reference argmin edge sits at
    slot 0 and the reference argmax edge at slot len-1. Destinations are
    sorted by degree and grouped into 128-dest blocks; each block gets its
    own padded segment length L (max cap in block, even, min 4; pads are
    copies of the argmax edge, so min/max stay exact and the sum is corrected
    by -pad*maxrow). This keeps total padding ~6% and makes every segment
    reduction a fixed-length contiguous reduction with static argmin/argmax
    row slices.
  - Device (per core): streams the padded fp16 edge buffer (SWDGE queue),
    computes per-dest min/max/sum via contiguous halving trees on the Vector
    engine (fp16 2x mode), corrects the sum for padding, assembles the
    440-wide stats block X, writes it to DRAM on the SWDGE queue (so edge
    prefetch and X writes never queue behind span-side DMAs), reads X back
    with xbar-transpose DMAs on the sync ring (1024-dest spans) to form X^T,
    and runs the 3-layer MLP with weights stationary, activations kept
    transposed [out_ch, dests]. The X DRAM buffer is an ExternalInput
    pre-filled by the host with the node features x at cols 440:472 and
    zeros elsewhere, so the transposed readback picks up x directly and no
    on-device merge or zero-fill DMA is needed. MLP chunks are emitted one span behind
    their transposes and stage-interleaved across a span's two 512-dest
    chunks, so Scalar RELUs of one chunk overlap the other chunk's matmuls
    and the PE never stalls on in-flight transposes. Tiny keep-warm matmuls
    prevent PE p-state downclocking between MLP bursts.
  - Host gathers per-core [32, S_pad] outputs, un-permutes to [50000, 32].
"""

import os
import time
from contextlib import ExitStack

import numpy as np

F16 = np.float16

N_NODES = 50000
N_EDGES = 800000
EC = 88
NC_DIM = 32
HC = 256
OC = 32
N_CORES = 8
S = N_NODES // N_CORES  # 6250
TARGET_GL = 160         # target dests-per-partition * L per super-tile
SPAN = 2048             # dests per transpose span

_LAST = {}  # debug/profiling stash for test.py


# ----------------------------------------------------------------------------
# Host-side preparation
# ----------------------------------------------------------------------------

def _host_prep(edge_index, edge_attr):
    dest = np.asarray(edge_index[1], np.int64)
    key = np.ascontiguousarray(np.asarray(edge_attr, np.float32)[:, 0])
    E = dest.shape[0]
    counts = np.bincount(dest, minlength=N_NODES)
    assert counts.min() >= 1, "empty destination segments not supported"

    order = np.argsort(dest, kind="stable")  # within dest: eid ascending
    starts = np.concatenate([[0], np.cumsum(counts)[:-1]]).astype(np.int64)
    k_s = key[order]
    segmax = np.maximum.reduceat(k_s, starts)
    segmin = np.minimum.reduceat(k_s, starts)
    dop = np.repeat(np.arange(N_NODES), counts)
    pos = np.arange(E, dtype=np.int64)
    argmax_pos = np.minimum.reduceat(np.where(k_s == segmax[dop], pos, E), starts)
    argmin_pos = np.minimum.reduceat(np.where(k_s == segmin[dop], pos, E), starts)

    cap = counts + (argmax_pos == argmin_pos)

    # Block-sorted binning: each core sorts its dests by cap ascending, pads
    # to a whole number of 128-dest blocks (pads first, cheapest L), and each
    # block gets its own L = max cap in block (across cores, since all cores
    # share one compiled program), rounded up to even, min 4.
    NB = (S + 127) // 128              # blocks per core
    S_pad = NB * 128
    npad_dests = S_pad - S
    plans = []
    block_max = np.zeros((N_CORES, NB), np.int64)
    for c in range(N_CORES):
        d0 = c * S
        dests = np.arange(d0, d0 + S)
        srt = dests[np.argsort(cap[d0:d0 + S], kind="stable")]
        slots = np.concatenate([np.full(npad_dests, -1, np.int64), srt])
        plans.append(slots)
        caps_sorted = np.concatenate(
            [np.zeros(npad_dests, np.int64), np.sort(cap[d0:d0 + S])])
        block_max[c] = caps_sorted.reshape(NB, 128).max(axis=1)

    L_of_block = np.maximum(4, ((block_max.max(axis=0) + 1) // 2) * 2)
    # runs of equal L -> bins
    L_values, n_b = [], []
    for L in L_of_block:
        if L_values and L_values[-1] == L:
            n_b[-1] += 128
        else:
            L_values.append(int(L))
            n_b.append(128)
    n_b = np.array(n_b, np.int64)
    E_pad = int(sum(L * n for L, n in zip(L_values, n_b)))

    return dict(L_values=L_values, n_b=n_b, S_pad=S_pad, E_pad=E_pad,
                order=order, starts=starts, counts=counts,
                argmax_pos=argmax_pos, argmin_pos=argmin_pos, plans=plans)


def _build_core_arrays(c, meta, x, edge_attr):
    L_values, n_b = meta["L_values"], meta["n_b"]
    order, starts, counts = meta["order"], meta["starts"], meta["counts"]
    argmax_pos, argmin_pos = meta["argmax_pos"], meta["argmin_pos"]
    dest_slots = meta["plans"][c]
    S_pad, E_pad = meta["S_pad"], meta["E_pad"]

    Ls = np.concatenate([np.full(n, L, np.int64) for L, n in zip(L_values, n_b)])
    row_start = np.concatenate([[0], np.cumsum(Ls)[:-1]])

    gidx = np.full(E_pad, -1, np.int64)
    real = dest_slots >= 0
    d_real = dest_slots[real]
    rs_real = row_start[real]
    L_real = Ls[real]
    cnt = counts[d_real]
    st = starts[d_real]
    apos = (argmax_pos[d_real] - st).astype(np.int64)
    ipos = (argmin_pos[d_real] - st).astype(np.int64)

    for i in range(d_real.shape[0]):
        s = st[i]; n = cnt[i]; L = L_real[i]; r0 = rs_real[i]
        seg = order[s:s + n]
        ia, ii = apos[i], ipos[i]
        if ia == ii:
            slots = np.concatenate([seg[ii:ii + 1], np.delete(seg, ii)])
        else:
            slots = np.concatenate(
                [seg[ii:ii + 1], np.delete(seg, [min(ia, ii), max(ia, ii)]),
                 seg[ia:ia + 1]])
        gidx[r0:r0 + n] = slots
        gidx[r0 + n:r0 + L] = seg[ia]

    edges = np.zeros((E_pad, EC), F16)
    valid = gidx >= 0
    edges[valid] = edge_attr[gidx[valid]].astype(F16)

    lens = np.where(real, counts[np.maximum(dest_slots, 0)], Ls).astype(np.float32)
    neg_pad = -(Ls - lens).astype(np.float32)
    neg_pad[~real] = 0.0
    inv_len = (1.0 / lens).astype(np.float32)

    xdram0 = np.zeros((S_pad, 512), F16)
    xdram0[real, 440:440 + NC_DIM] = np.asarray(x, np.float32)[d_real].astype(F16)
    NT = S_pad // 128

    return dict(
        edges=edges,
        negpad=np.ascontiguousarray(
            np.broadcast_to(neg_pad[:, None], (S_pad, EC)).reshape(
                NT, 128, EC).transpose(1, 0, 2)).astype(F16),
        invlen=np.ascontiguousarray(
            np.broadcast_to(inv_len[:, None], (S_pad, EC)).reshape(
                NT, 128, EC).transpose(1, 0, 2)).astype(F16),
        xdram0=xdram0,
        dest_slots=dest_slots,
    )


def _pack_weights(W1, b1, W2, b2, W3, b3):
    W1 = np.asarray(W1, np.float32); W2 = np.asarray(W2, np.float32)
    W3 = np.asarray(W3, np.float32)
    b1 = np.asarray(b1, np.float32); b2 = np.asarray(b2, np.float32)
    b3 = np.asarray(b3, np.float32)
    # reorder W1 rows: stats first (minrow maxrow mn mean mx), then x
    W1r = np.concatenate([W1[NC_DIM:], W1[:NC_DIM]], axis=0)  # [472, 256]
    W1p = np.zeros((512, HC), np.float32)
    W1p[:472] = W1r
    w1 = np.ascontiguousarray(
        W1p.reshape(4, 128, HC).transpose(1, 0, 2)).astype(F16)   # [128,4,256]
    w2 = np.ascontiguousarray(
        W2.reshape(2, 128, HC).transpose(1, 0, 2)).astype(F16)    # [128,2,256]
    w3 = np.ascontiguousarray(
        W3.reshape(2, 128, OC).transpose(1, 0, 2)).astype(F16)    # [128,2,32]
    b1p = np.ascontiguousarray(b1.reshape(2, 128).T)               # [128,2]
    b2p = np.ascontiguousarray(b2.reshape(2, 128).T)
    b3p = np.zeros((128, 1), np.float32)
    b3p[:OC, 0] = b3
    ident = np.eye(128, dtype=F16)
    return dict(w1=w1, w2=w2, w3=w3, b1=b1p, b2=b2p, b3=b3p, ident=ident)


# ----------------------------------------------------------------------------
# Device program
# ----------------------------------------------------------------------------

def _super_tiles(meta):
    """Static schedule: list of (L, G, dest0, row0)."""
    sts = []
    dest0 = 0
    row0 = 0
    for L, n in zip(meta["L_values"], meta["n_b"]):
        G_full = max(1, TARGET_GL // L)
        blocks = n // 128
        pos = 0
        while pos < blocks:
            g = min(G_full, blocks - pos)
            # graduated pipeline ramp: tiny first tiles so the Vector engine
            # starts as soon as possible instead of waiting on one big DMA
            if len(sts) == 0:
                g = min(g, 2)
            elif len(sts) == 1:
                g = min(g, 4)
            elif len(sts) == 2:
                g = min(g, 8)
            sts.append((L, g, dest0 + pos * 128, row0 + pos * 128 * L))
            pos += g
        dest0 += n
        row0 += n * L
    return sts


def _build_program(meta):
    import concourse.bass as bass
    import concourse.tile as tile
    from concourse import bacc, mybir

    f32 = mybir.dt.float32
    f16 = mybir.dt.float16
    Alu = mybir.AluOpType
    Act = mybir.ActivationFunctionType

    S_pad, E_pad = meta["S_pad"], meta["E_pad"]
    NT = S_pad // 128
    NBLK = S_pad // 128
    sts = _super_tiles(meta)

    nc = bacc.Bacc("TRN2", target_bir_lowering=False, debug=False,
                   num_devices=N_CORES)

    edges_h = nc.dram_tensor("edges", [E_pad, EC], f16, kind="ExternalInput")
    np_h = nc.dram_tensor("negpad", [128, NT, EC], f16, kind="ExternalInput")
    il_h = nc.dram_tensor("invlen", [128, NT, EC], f16, kind="ExternalInput")
    w1_h = nc.dram_tensor("w1", [128, 4, HC], f16, kind="ExternalInput")
    w2_h = nc.dram_tensor("w2", [128, 2, HC], f16, kind="ExternalInput")
    w3_h = nc.dram_tensor("w3", [128, 2, OC], f16, kind="ExternalInput")
    b1_h = nc.dram_tensor("b1", [128, 2], f32, kind="ExternalInput")
    b2_h = nc.dram_tensor("b2", [128, 2], f32, kind="ExternalInput")
    b3_h = nc.dram_tensor("b3", [128, 1], f32, kind="ExternalInput")
    xdram_h = nc.dram_tensor("xdram", [S_pad, 512], f16, kind="ExternalInput")
    outT_h = nc.dram_tensor("outT", [OC, S_pad], f32, kind="ExternalOutput")

    with tile.TileContext(nc) as tc, ExitStack() as ctx:
        singles = ctx.enter_context(tc.tile_pool(name="singles", bufs=1))
        epool = ctx.enter_context(tc.tile_pool(name="edg", bufs=3))
        tpool = ctx.enter_context(tc.tile_pool(name="tree", bufs=1))
        xpool = ctx.enter_context(tc.tile_pool(name="xrows", bufs=2))
        xtp = ctx.enter_context(tc.tile_pool(name="xtq", bufs=2))
        hp = ctx.enter_context(tc.tile_pool(name="hid", bufs=3))
        opool = ctx.enter_context(tc.tile_pool(name="outb", bufs=2))
        pmm = ctx.enter_context(tc.tile_pool(name="psmm", bufs=4, space="PSUM"))
        pwarm = ctx.enter_context(tc.tile_pool(name="pswm", bufs=1, space="PSUM"))

        # --- load constants (npad/ilen first: the first tile's mean ops
        # need them; scalar ring so they don't queue behind weights) ---
        npad_sb = singles.tile([128, NT, EC], f16)
        nc.scalar.dma_start(out=npad_sb, in_=np_h.ap())
        ilen_sb = singles.tile([128, NT, EC], f16)
        nc.scalar.dma_start(out=ilen_sb, in_=il_h.ap())
        w1_sb = singles.tile([128, 4, HC], f16)
        nc.sync.dma_start(out=w1_sb, in_=w1_h.ap())
        w2_sb = singles.tile([128, 2, HC], f16)
        nc.sync.dma_start(out=w2_sb, in_=w2_h.ap())
        w3_sb = singles.tile([128, 2, OC], f16)
        nc.sync.dma_start(out=w3_sb, in_=w3_h.ap())
        b1_sb = singles.tile([128, 2], f32)
        nc.sync.dma_start(out=b1_sb, in_=b1_h.ap())
        b2_sb = singles.tile([128, 2], f32)
        nc.sync.dma_start(out=b2_sb, in_=b2_h.ap())
        b3_sb = singles.tile([128, 1], f32)
        nc.sync.dma_start(out=b3_sb, in_=b3_h.ap())

        def bcast(ap2d, n):
            return bass.AP(tensor=ap2d.tensor, offset=ap2d.offset,
                           ap=[*ap2d.ap, [0, n]])

        def emit_tree(et4, L, G, op, final_out):
            h = L // 2
            T = tpool.tile([128, G, h * EC], f16, tag="tree", name="tree")
            T4 = T[:].rearrange("p g (m e) -> p g m e", e=EC)
            nc.vector.tensor_tensor(out=T4[:, :, 0:h, :], in0=et4[:, :, 0:h, :],
                                    in1=et4[:, :, h:2 * h, :], op=op)
            m = h
            while m > 1:
                if m % 2 == 1:
                    nc.vector.tensor_tensor(out=T4[:, :, 0, :],
                                            in0=T4[:, :, 0, :],
                                            in1=T4[:, :, m - 1, :], op=op)
                    m -= 1
                h2 = m // 2
                if h2 == 1:
                    nc.vector.tensor_tensor(out=final_out, in0=T4[:, :, 0, :],
                                            in1=T4[:, :, 1, :], op=op)
                else:
                    nc.vector.tensor_tensor(out=T4[:, :, 0:h2, :],
                                            in0=T4[:, :, 0:h2, :],
                                            in1=T4[:, :, h2:m, :], op=op)
                m = h2

        blocks_ready = 0
        spans_emitted = 0
        # span boundaries: SPAN-sized, but carve the final 256 dests into a
        # tiny last span so the pipeline tail is short
        bounds = [0, 512, 1024] + list(range(1024 + SPAN, S_pad, SPAN)) + [S_pad]
        bounds = sorted(set(b for b in bounds if b <= S_pad))
        if S_pad - bounds[-2] > 256:
            bounds.insert(-1, S_pad - 256)
        spans = [(bounds[i], bounds[i + 1] - bounds[i])
                 for i in range(len(bounds) - 1)]
        NSPAN = len(spans)
        xt_span = [None] * 4
        span_d0 = [0]
        pending_spans = []

        def emit_span(si):
            d0, nd = spans[si]
            parts = [128, 128, 128, 88]
            for k in range(4):
                xt_t = xtp.tile([128, SPAN], f16, tag=f"xt{k}", name=f"xt{k}")
                nc.sync.dma_start(
                    out=xt_t[:, 0:nd],
                    in_=xdram_h.ap()[d0:d0 + nd, k * 128:(k + 1) * 128],
                    transpose=True)
                xt_span[k] = xt_t
            span_d0[0] = d0

        def emit_span_chunks(sd0, snd, xts):
            """MLP for one span, stage-interleaved across its 512-dest chunks
            so each chunk's RELUs overlap the other chunk's matmuls."""
            parts = [128, 128, 128, 88]
            chunks = [(sd0 + coff, min(512, snd - coff))
                      for coff in range(0, snd, 512)]
            h1s_of, h2s_of = {}, {}
            for (cd0, N) in chunks:
                off = cd0 - sd0
                h1s = []
                for cc in range(2):
                    ps = pmm.tile([128, 512], f32, tag="mm", name="mm")
                    for k in range(4):
                        P = parts[k]
                        nc.tensor.matmul(ps[:, 0:N],
                                         w1_sb[0:P, k, cc * 128:(cc + 1) * 128],
                                         xts[k][0:P, off:off + N],
                                         start=(k == 0), stop=(k == 3))
                    h1 = hp.tile([128, 512], f16, tag=f"h1{cc}", name=f"h1{cc}")
                    nc.scalar.activation(out=h1[:, 0:N], in_=ps[:, 0:N],
                                         func=Act.Relu, bias=b1_sb[:, cc:cc + 1],
                                         scale=1.0)
                    h1s.append(h1)
                h1s_of[cd0] = h1s
            for (cd0, N) in chunks:
                h1s = h1s_of[cd0]
                h2s = []
                for cc in range(2):
                    ps = pmm.tile([128, 512], f32, tag="mm", name="mm")
                    for k in range(2):
                        nc.tensor.matmul(ps[:, 0:N],
                                         w2_sb[:, k, cc * 128:(cc + 1) * 128],
                                         h1s[k][:, 0:N],
                                         start=(k == 0), stop=(k == 1))
                    h2 = hp.tile([128, 512], f16, tag=f"h2{cc}", name=f"h2{cc}")
                    nc.scalar.activation(out=h2[:, 0:N], in_=ps[:, 0:N],
                                         func=Act.Relu, bias=b2_sb[:, cc:cc + 1],
                                         scale=1.0)
                    h2s.append(h2)
                h2s_of[cd0] = h2s
            for (cd0, N) in chunks:
                h2s = h2s_of[cd0]
                ps3 = pmm.tile([32, 512], f32, tag="mm3", name="mm3", bufs=2)
                for k in range(2):
                    nc.tensor.matmul(ps3[:, 0:N], w3_sb[:, k, :],
                                     h2s[k][:, 0:N], start=(k == 0), stop=(k == 1))
                ob = opool.tile([32, 512], f32, tag="ob", name="ob")
                nc.scalar.activation(out=ob[:, 0:N], in_=ps3[:, 0:N],
                                     func=Act.Identity, bias=b3_sb[0:OC, :],
                                     scale=1.0)
                nc.scalar.dma_start(out=outT_h.ap()[:, cd0:cd0 + N],
                                    in_=ob[:, 0:N])

        for (L, G, dest0, row0) in sts:
            et = epool.tile([128, G, L * EC], f16, tag="et", name="et")
            src_ap = edges_h.ap()[row0:row0 + 128 * G * L, :].rearrange(
                "(g p l) e -> p g (l e)", g=G, p=128)
            nc.gpsimd.dma_start(out=et, in_=src_ap)
            et4 = et[:].rearrange("p g (l e) -> p g l e", e=EC)

            dsl = xdram_h.ap()[dest0:dest0 + 128 * G, :]
            xt = xpool.tile([128, G, 440], f16, tag="xt", name="xt")
            # minrow / maxrow (slots 0 and L-1) copied on the Scalar engine
            nc.scalar.copy(out=xt[:, :, 0:EC], in_=et4[:, :, 0, :])
            nc.scalar.copy(out=xt[:, :, EC:2 * EC], in_=et4[:, :, L - 1, :])
            # trees: mn -> xt[176:264], mx -> xt[352:440]
            emit_tree(et4, L, G, Alu.min, xt[:, :, 2 * EC:3 * EC])
            emit_tree(et4, L, G, Alu.max, xt[:, :, 4 * EC:5 * EC])
            sm = tpool.tile([128, G, EC], f16, tag="sm", name="sm")
            emit_tree(et4, L, G, Alu.add, sm[:, :, :])
            # sum correction + mean -> xt[88:176] (all fp16, 2x mode)
            c0 = dest0 // 128
            t1 = tpool.tile([128, G, EC], f16, tag="t1", name="t1")
            nc.vector.tensor_tensor(out=t1[:, :, :], in0=et4[:, :, L - 1, :],
                                    in1=npad_sb[:, c0:c0 + G, :],
                                    op=Alu.mult)
            nc.vector.tensor_tensor(out=sm[:, :, :], in0=sm[:, :, :],
                                    in1=t1[:, :, :], op=Alu.add)
            nc.vector.tensor_tensor(out=xt[:, :, 3 * EC:4 * EC],
                                    in0=sm[:, :, :],
                                    in1=ilen_sb[:, c0:c0 + G, :],
                                    op=Alu.mult)
            # PE keep-warm pulses: tiny matmuls reading freshly written stats
            for woff in (0, EC, 2 * EC):
                wps = pwarm.tile([64, 88], f32, tag="warm", name="warm")
                nc.tensor.matmul(wps, xt[:, 0, woff:woff + 64],
                                 xt[:, 0, woff:woff + EC],
                                 start=True, stop=True)
            # X rows (cols 0:440) to X DRAM in one DMA (SWDGE queue, so it
            # never waits behind span transpose reads on the sync ring)
            nc.gpsimd.dma_start(
                out=dsl[:, 0:5 * EC].rearrange("(g p) c -> p g c", p=128),
                in_=xt)
            blocks_ready = dest0 // 128 + G
            while spans_emitted < NSPAN and \
                    (spans[spans_emitted][0] + spans[spans_emitted][1]
                     <= blocks_ready * 128):
                # transposes now; MLP chunks one span later, so the PE never
                # reaches a span's matmuls before its transposes complete
                emit_span(spans_emitted)
                if pending_spans:
                    sd0, snd, xts = pending_spans.pop(0)
                    emit_span_chunks(sd0, snd, xts)
                sd0, snd = spans[spans_emitted]
                pending_spans.append((sd0, snd, list(xt_span)))
                spans_emitted += 1
        for (sd0, snd, xts) in pending_spans:
            emit_span_chunks(sd0, snd, xts)

    nc.finalize()
    return nc


# ----------------------------------------------------------------------------
# Entry point
# ----------------------------------------------------------------------------

def kernel(x, edge_index, edge_attr, u, batch, W1, b1, W2, b2, W3, b3,
           **_unused):
    t0 = time.time()
    x = np.asarray(x)
    edge_index = np.asarray(edge_index)
    edge_attr = np.asarray(edge_attr, np.float32)

    meta = _host_prep(edge_index, edge_attr)
    packed = _pack_weights(W1, b1, W2, b2, W3, b3)
    cores = [_build_core_arrays(c, meta, x, edge_attr) for c in range(N_CORES)]
    t1 = time.time()

    nc = _build_program(meta)
    t2 = time.time()

    in_maps = []
    for c in range(N_CORES):
        m = dict(edges=cores[c]["edges"], xdram=cores[c]["xdram0"],
                 negpad=cores[c]["negpad"], invlen=cores[c]["invlen"],
                 w1=packed["w1"], w2=packed["w2"], w3=packed["w3"],
                 b1=packed["b1"], b2=packed["b2"], b3=packed["b3"])
        in_maps.append(m)

    from concourse.bass_utils import run_bass_kernel_spmd
    trace = os.environ.get("NNK_TRACE", "0") == "1"
    res = run_bass_kernel_spmd(nc, in_maps, core_ids=list(range(N_CORES)),
                               trace=trace)
    t3 = time.time()

    out = np.zeros((N_NODES, OC), np.float32)
    for c in range(N_CORES):
        outT = res.results[c]["outT"]
        ds = cores[c]["dest_slots"]
        real = ds >= 0
        out[ds[real]] = outT[:, real].T
    _LAST.update(nc=nc, meta=meta, in_maps=in_maps, res=res,
                 host_s=t1 - t0, build_s=t2 - t1, run_s=t3 - t2)
    return out

